# revision 34
# baseline (speedup 1.0000x reference)
"""Dual-branch multi-head attention on 8 Trainium2 NeuronCores.

Problem (B, S, D, H, DH) = (4, 1024, 1024, 16, 64):
    q/k/v + sq/sk/sv projections of x, two softmax attentions, weighted sum.

Sharding: tensor-parallel over heads — core c owns heads {2c, 2c+1} of both
branches (output columns 128c..128c+128). Each core reads the full x
(pre-transposed on host to xT [D, B*S]) and its [D, 128] weight slices.
No collectives: host concatenates per-core outputs along the feature axis.

Per-core pipeline (all layouts transposed, features on partitions):
  proj:   qT = Wq^T @ xT  (PSUM accum over 8 k-chunks, bias fused in the
          PSUM->SBUF copy). Scale 1/sqrt(DH) folded into Wq on host;
          combine weights softmax(attn_w) folded into Wv/Wsv on host.
  v_aug:  PE-transpose vT -> v natural [token, dh], append a ones column
          (denominator trick).
  scores: scoresT[j,i] = k^T.T @ qT with contraction DH=64; the two heads
          run as concurrent row-tiled matmuls (array rows 0-63 / 64-127).
  exp:    ACT PSUM->SBUF, no max subtraction (scores ~ N(0,1), exp safe).
  PV:     ctx_augT[65, i] = v_aug.T @ probsT accumulated over j-chunks;
          row 64 = softmax denominator (ones column).
  norm:   DVE reciprocal of denominator row + partition-broadcast multiply.
  out:    combine branches, PE-transpose back to [token, dh], DMA out.

Matmul dtype float32r (TF32-like, 4x faster than fp32 on trn2 PE);
producers write fp32r tiles directly (HW rounds on write).
"""

import os
import numpy as np

import concourse.bass as bass
import concourse.bacc as bacc
import concourse.tile as tile
from concourse import mybir
from concourse.bass_utils import run_bass_kernel_spmd

dt = mybir.dt
Alu = mybir.AluOpType
Act = mybir.ActivationFunctionType

B, S, D, H, DH = 4, 1024, 1024, 16, 64
NCORES = 8
HPC = H // NCORES            # heads per core = 2
CW = HPC * DH                # output cols per core = 128
KC = D // 128                # contraction chunks = 8
JC = S // 128                # key-token chunks = 8
NIC = S // 512               # query i-chunks of 512 = 2
NTB = (B * S) // B // 512    # token blocks per batch = 2

MM = {"f32r": dt.float32r, "f32": dt.float32, "bf16": dt.bfloat16}[
    os.environ.get("KMM", "bf16")
]

PROJ = ["q", "k", "v", "sq", "sk", "sv"]


def _emit(nc, tc, ctx, prm):
    """Emit the whole kernel under TileContext tc. prm: dram param handles."""
    f32 = dt.float32
    bf16 = dt.bfloat16

    env = lambda k, d: int(os.environ.get(k, d))
    const = ctx.enter_context(tc.tile_pool(name="const", bufs=1))
    xpool = ctx.enter_context(tc.tile_pool(name="xp", bufs=env("KXB", 4 * KC + 1)))
    popool = ctx.enter_context(tc.tile_pool(name="po", bufs=env("KPO", 2)))
    vapool = ctx.enter_context(tc.tile_pool(name="va", bufs=env("KVA", 8)))
    prpool = ctx.enter_context(tc.tile_pool(name="pr", bufs=env("KPR", 4)))
    nmpool = ctx.enter_context(tc.tile_pool(name="nm", bufs=env("KNM", 9)))
    rcpool = ctx.enter_context(tc.tile_pool(name="rc", bufs=2))
    oupool = ctx.enter_context(tc.tile_pool(name="ou", bufs=2))
    # PSUM budget (8 banks): proj-pair/otp slots 2x1, score blocks 2x2,
    # PV accumulators 2x1.
    ps_mm = ctx.enter_context(tc.tile_pool(name="psmm", bufs=2, space="PSUM"))
    ps_sc = ctx.enter_context(tc.tile_pool(name="pssc", bufs=2, space="PSUM"))
    ps_cx = ctx.enter_context(tc.tile_pool(name="pscx", bufs=2, space="PSUM"))

    # constants: weights (fp32r, [128, KC*128] with k-chunk c at cols 128c),
    # biases [128, 1] (v/sv bias folded into obias), identity [128, 64]
    wt, bt = {}, {}

    def load_consts(names):
        # sync HWDGE queue for the simple 2D bias loads; weights keep the
        # known-good gpsimd SWDGE path (3D gather pattern).
        for p in names:
            wt[p] = const.tile([128, KC * 128], MM, tag=f"w_{p}", name=f"w_{p}")
            nc.gpsimd.dma_start(
                out=wt[p].rearrange("p (c n) -> p c n", n=128),
                in_=prm[f"W{p}"].rearrange("(c p) n -> p c n", p=128),
            )
            if p in ("v", "sv"):
                continue
            bt[p] = const.tile([128, 1], f32, tag=f"b_{p}", name=f"b_{p}")
            nc.sync.dma_start(out=bt[p][:], in_=prm[f"b{p}"][:, None])

    load_consts(["q", "k"])
    ident = const.tile([128, DH], f32, tag="ident", name="ident")
    nc.sync.dma_start(out=ident[:], in_=prm["ident"][:])
    identb = const.tile([128, DH], bf16, tag="identb", name="identb")
    nc.vector.tensor_copy(identb[:], ident[:])
    obias = const.tile([128, 8 * CW], f32, tag="obias", name="obias")
    nc.sync.dma_start(out=obias[:], in_=prm["obias"][:])

    # per-batch state handed from proj gen to attn gen
    projT = [None] * B   # dict p -> [128, S] tile (qT/kT/sqT/skT fp32r, vT/svT f32)
    vaug = [None] * B    # dict (br, h) -> [128, JC*65] fp32r tile

    def issue_x(b):
        # SWDGE (Pool) queue, parallel to the sync HWDGE queue; issued for
        # all batches upfront so the PE is never x-starved.
        xt = []
        for kc in range(KC):
            t = xpool.tile([128, S], MM, tag="xt", name="xt")
            nc.gpsimd.dma_start(
                out=t[:], in_=prm["xT"][128 * kc : 128 * (kc + 1), S * b : S * (b + 1)]
            )
            xt.append(t)
        return xt

    def gen_proj(b, xt):
        pj = {}
        for p in PROJ:
            pj[p] = popool.tile([128, S], MM, tag=f"pj_{p}", name=f"pj_{p}")
        projT[b] = pj
        for tb in range(NTB):
            for pair in (("q", "k"), ("v", "sq"), ("sk", "sv")):
                ps = {p: ps_mm.tile([128, 512], f32, tag="pmm", name=f"ps_{p}")[:] for p in pair}
                for kc in range(KC):
                    for p in pair:
                        nc.tensor.matmul(
                            ps[p],
                            wt[p][:, 128 * kc : 128 * (kc + 1)],
                            xt[kc][:, 512 * tb : 512 * (tb + 1)],
                            start=(kc == 0),
                            stop=(kc == KC - 1),
                        )
                    yield
                for p in pair:
                    if p in ("v", "sv"):
                        # bias folded into obias; cast f32 PSUM -> bf16 SBUF
                        nc.vector.tensor_copy(
                            pj[p][:, 512 * tb : 512 * (tb + 1)], ps[p]
                        )
                    else:
                        nc.vector.tensor_scalar_add(
                            pj[p][:, 512 * tb : 512 * (tb + 1)], ps[p], bt[p][:]
                        )
        # v natural + ones column (denominator trick); PE transpose in bf16
        va = {}
        vaug[b] = va
        for br in range(2):
            vsrc = pj["v" if br == 0 else "sv"]
            for h in range(HPC):
                t = vapool.tile([128, JC * (DH + 1)], bf16, tag="vaug", name="vaug")
                va[br, h] = t
                tv = t.rearrange("p (c w) -> p c w", w=DH + 1)
                nc.vector.memset(tv[:, :, DH], 1.0)
                tp = ps_mm.tile([128, 512], bf16, tag="pmm", name="vtp")
                for jc in range(JC):
                    nc.tensor.transpose(
                        tp[:, DH * jc : DH * (jc + 1)],
                        vsrc[DH * h : DH * (h + 1), 128 * jc : 128 * (jc + 1)],
                        identb[DH * h : DH * (h + 1), :],
                    )
                yield
                nc.vector.tensor_copy(
                    tv[:, :, 0:DH], tp.rearrange("p (c w) -> p c w", w=DH)
                )
                yield

    def gen_attn(b):
        pj = projT[b]
        va = vaug[b]
        normed = {}
        for br in range(2):
            qT = pj["q" if br == 0 else "sq"]
            kT = pj["k" if br == 0 else "sk"]
            for ic in range(NIC):
                cx = {h: ps_cx.tile([128, 512], f32, tag="ctx", name=f"cx{h}") for h in range(HPC)}
                for jp in range(JC // 2):
                    pr = {}
                    for h in range(HPC):
                        sc = ps_sc.tile([128, 1024], f32, tag="sc", name="sc")
                        for half in range(2):
                            jc = 2 * jp + half
                            nc.tensor.matmul(
                                sc[:, 512 * half : 512 * (half + 1)],
                                kT[DH * h : DH * (h + 1), 128 * jc : 128 * (jc + 1)],
                                qT[DH * h : DH * (h + 1), 512 * ic : 512 * (ic + 1)],
                                start=True,
                                stop=True,
                            )
                        p = prpool.tile([128, 1024], bf16, tag="probs", name="probs")
                        nc.scalar.activation(p[:], sc[:], Act.Exp)
                        pr[h] = p
                    yield
                    for h in range(HPC):
                        for half in range(2):
                            jc = 2 * jp + half
                            nc.tensor.matmul(
                                cx[h][0 : DH + 1, :],
                                va[br, h][:, (DH + 1) * jc : (DH + 1) * (jc + 1)],
                                pr[h][:, 512 * half : 512 * (half + 1)],
                                start=(jc == 0),
                                stop=(jc == JC - 1),
                            )
                    yield
                for h in range(HPC):
                    rcp = rcpool.tile([1, 512], dt.float32, tag="rcp", name="rcp")
                    nc.vector.reciprocal(rcp[:], cx[h][DH : DH + 1, :])
                    rcpb = rcpool.tile([DH, 512], dt.float32, tag="rcpb", name="rcpb")
                    nc.gpsimd.partition_broadcast(rcpb[:], rcp[:])
                    nt = nmpool.tile([DH, 512], dt.float32, tag="normed", name="normed")
                    nc.vector.tensor_tensor(nt[:], cx[h][0:DH, :], rcpb[:], Alu.mult)
                    normed[br, h, ic] = nt
        outsb = oupool.tile([128, 8 * CW], dt.float32, tag="outsb", name="outsb")
        ov = outsb.rearrange("p (c w) -> p c w", w=CW)
        for h in range(HPC):
            tp = ps_mm.tile([128, 512], dt.float32, tag="pmm", name="otp")
            for ic in range(NIC):
                cb = nmpool.tile([DH, 512], dt.float32, tag="comb", name="comb", bufs=3)
                nc.vector.tensor_add(cb[:], normed[0, h, ic][:], normed[1, h, ic][:])
                for icc in range(4):
                    nc.tensor.transpose(
                        tp[:, DH * (4 * ic + icc) : DH * (4 * ic + icc + 1)],
                        cb[:, 128 * icc : 128 * (icc + 1)],
                        ident[0:DH, :],
                    )
            nc.vector.tensor_tensor(
                ov[:, :, DH * h : DH * (h + 1)],
                tp.rearrange("p (c w) -> p c w", w=DH),
                obias.rearrange("p (c w) -> p c w", w=CW)[:, :, DH * h : DH * (h + 1)],
                Alu.add,
            )
            yield
        nc.sync.dma_start(
            out=prm["out"][b].rearrange("(c p) d -> p c d", p=128),
            in_=outsb.rearrange("p (c d) -> p c d", d=CW)
        )

    # driver: software-pipeline proj(b+1) into attention(b)'s rounds so the
    # PE always has dense matmul work while ACT chews through the exps.
    # KREP repeats the whole pipeline in-NEFF (timing: slope vs rep count).
    first_rep = True
    for rep in range(int(os.environ.get("KREP", "1"))):
        # queue order on the Pool SWDGE ring is program order: batch-0 x
        # first, then the remaining weights, then the other batches' x.
        xts = [issue_x(0)]
        if first_rep:
            load_consts(["v", "sq", "sk", "sv"])
            first_rep = False
        xts += [issue_x(b) for b in range(1, B)]
        pgens = [gen_proj(b, xts[b]) for b in range(B)]
        for _ in pgens[0]:
            pass
        for b in range(B):
            pg = pgens[b + 1] if b + 1 < B else None
            for _ in gen_attn(b):
                if pg is not None:
                    for _ in range(int(os.environ.get("KPULL", "1"))):
                        if next(pg, "done") == "done":
                            pg = None
                            break
            while pg is not None and next(pg, "done") != "done":
                pass


def build_nc():
    nc = bacc.Bacc("TRN2", target_bir_lowering=False, debug=False)
    prm = {}
    prm["xT"] = nc.declare_dram_parameter("xT", [D, B * S], MM, isOutput=False)
    for p in PROJ:
        prm[f"W{p}"] = nc.declare_dram_parameter(f"W{p}", [D, CW], MM, isOutput=False)
        if p not in ("v", "sv"):
            prm[f"b{p}"] = nc.declare_dram_parameter(f"b{p}", [CW], dt.float32, isOutput=False)
    prm["ident"] = nc.declare_dram_parameter("ident", [128, DH], dt.float32, isOutput=False)
    prm["obias"] = nc.declare_dram_parameter("obias", [128, 8 * CW], dt.float32, isOutput=False)
    prm["out"] = nc.declare_dram_parameter("out", [B, S, CW], dt.float32, isOutput=True)

    from contextlib import ExitStack

    with tile.TileContext(nc) as tc:
        with ExitStack() as ctx:
            _emit(nc, tc, ctx, prm)
    nc.compile()
    return nc


def make_in_maps(hidden_states, Wq, bq, Wk, bk, Wv, bv, Wsq, bsq, Wsk, bsk, Wsv, bsv, attn_w):
    """Host-side sharding: slice per-head weight columns, fold scales."""
    f32 = np.float32
    mmnp = dt.np(MM)
    x = np.asarray(hidden_states, f32).reshape(B * S, D)
    xT = np.ascontiguousarray(x.T).astype(mmnp)
    a = np.asarray(attn_w, f32)
    e = np.exp(a - a.max())
    w = (e / e.sum()).astype(f32)
    sc = f32(1.0 / np.sqrt(DH))
    ident = np.tile(np.eye(DH, dtype=f32), (2, 1))

    full = {
        "q": (np.asarray(Wq, f32) * sc, np.asarray(bq, f32) * sc),
        "k": (np.asarray(Wk, f32), np.asarray(bk, f32)),
        "v": (np.asarray(Wv, f32) * w[0], None),
        "sq": (np.asarray(Wsq, f32) * sc, np.asarray(bsq, f32) * sc),
        "sk": (np.asarray(Wsk, f32), np.asarray(bsk, f32)),
        "sv": (np.asarray(Wsv, f32) * w[1], None),
    }
    # v/sv bias folds linearly through attention into a constant output bias
    comb_bias = (w[0] * np.asarray(bv, f32) + w[1] * np.asarray(bsv, f32)).astype(f32)
    in_maps = []
    for c in range(NCORES):
        cols = slice(CW * c, CW * (c + 1))
        m = {"xT": xT, "ident": ident}
        m["obias"] = np.ascontiguousarray(
            np.tile(comb_bias[cols], (128, 8)).astype(f32)
        )
        for p in PROJ:
            W, b = full[p]
            m[f"W{p}"] = np.ascontiguousarray(W[:, cols]).astype(mmnp)
            if b is not None:
                m[f"b{p}"] = np.ascontiguousarray(b[cols])
        in_maps.append(m)
    return in_maps


_NC_CACHE = {}


def get_nc():
    if "nc" not in _NC_CACHE:
        _NC_CACHE["nc"] = build_nc()
    return _NC_CACHE["nc"]


def kernel(**inputs):
    nc = get_nc()
    in_maps = make_in_maps(**inputs)
    res = run_bass_kernel_spmd(nc, in_maps, list(range(NCORES)))
    parts = [res.results[c]["out"] for c in range(NCORES)]
    return np.concatenate(parts, axis=2).astype(np.float32)



# revision 36
# speedup vs baseline: 246.8307x; 246.8307x over previous
"""Dual-branch multi-head attention on 8 Trainium2 NeuronCores.

Problem (B, S, D, H, DH) = (4, 1024, 1024, 16, 64):
    q/k/v + sq/sk/sv projections of x, two softmax attentions, weighted sum.

Sharding: tensor-parallel over heads — core c owns heads {2c, 2c+1} of both
branches (output columns 128c..128c+128). Each core reads the full x
(pre-transposed on host to xT [D, B*S]) and its [D, 128] weight slices.
No collectives: host concatenates per-core outputs along the feature axis.

Per-core pipeline (all layouts transposed, features on partitions):
  proj:   qT = Wq^T @ xT  (PSUM accum over 8 k-chunks, bias fused in the
          PSUM->SBUF copy). Scale 1/sqrt(DH) folded into Wq on host;
          combine weights softmax(attn_w) folded into Wv/Wsv; v/sv biases
          fold linearly through attention into one output bias (obias),
          applied by the final PSUM->SBUF add.
  v_aug:  PE-transpose vT -> v natural [token, dh] (bf16), append a ones
          column (denominator trick).
  scores: scoresT[j,i] = k^T.T @ qT with contraction DH=64; the two heads
          run as concurrent row-tiled matmuls (array rows 0-63 / 64-127).
  exp:    ACT PSUM->SBUF bf16, no max subtraction (scores ~ N(0,1)).
  PV:     ctx_augT[65, i] = v_aug.T @ probsT accumulated over j-chunks;
          row 64 = softmax denominator (ones column).
  norm:   DVE reciprocal of denominator row + partition-broadcast multiply.
  out:    combine branches, PE-transpose back to [token, dh], add obias,
          DMA out.

Matmul/x/weight dtype bf16 (same PE throughput as fp32r, half the DMA and
SBUF footprint; rel err ~5e-3 vs the 2e-2 gate). x for all 4 batches is
prefetched upfront on the Pool SWDGE queue, ordered so batch-0 x and all
weights land first; proj(b+1) is software-pipelined into attention(b).
"""

import os
import numpy as np

import concourse.bass as bass
import concourse.bacc as bacc
import concourse.tile as tile
from concourse import mybir
from concourse.bass_utils import run_bass_kernel_spmd

dt = mybir.dt
Alu = mybir.AluOpType
Act = mybir.ActivationFunctionType

B, S, D, H, DH = 4, 1024, 1024, 16, 64
NCORES = 8
HPC = H // NCORES            # heads per core = 2
CW = HPC * DH                # output cols per core = 128
KC = D // 128                # contraction chunks = 8
JC = S // 128                # key-token chunks = 8
NIC = S // 512               # query i-chunks of 512 = 2
NTB = (B * S) // B // 512    # token blocks per batch = 2

MM = {"f32r": dt.float32r, "f32": dt.float32, "bf16": dt.bfloat16}[
    os.environ.get("KMM", "bf16")
]

PROJ = ["q", "k", "v", "sq", "sk", "sv"]


def _emit(nc, tc, ctx, prm):
    """Emit the whole kernel under TileContext tc. prm: dram param handles."""
    f32 = dt.float32
    bf16 = dt.bfloat16

    env = lambda k, d: int(os.environ.get(k, d))
    const = ctx.enter_context(tc.tile_pool(name="const", bufs=1))
    xpool = ctx.enter_context(tc.tile_pool(name="xp", bufs=env("KXB", 4 * KC + 1)))
    popool = ctx.enter_context(tc.tile_pool(name="po", bufs=env("KPO", 2)))
    vapool = ctx.enter_context(tc.tile_pool(name="va", bufs=env("KVA", 8)))
    prpool = ctx.enter_context(tc.tile_pool(name="pr", bufs=env("KPR", 4)))
    nmpool = ctx.enter_context(tc.tile_pool(name="nm", bufs=env("KNM", 9)))
    rcpool = ctx.enter_context(tc.tile_pool(name="rc", bufs=2))
    oupool = ctx.enter_context(tc.tile_pool(name="ou", bufs=2))
    # PSUM budget (8 banks): proj-pair/otp slots 2x1, score blocks 2x2,
    # PV accumulators 2x1.
    ps_mm = ctx.enter_context(tc.tile_pool(name="psmm", bufs=2, space="PSUM"))
    ps_sc = ctx.enter_context(tc.tile_pool(name="pssc", bufs=2, space="PSUM"))
    ps_cx = ctx.enter_context(tc.tile_pool(name="pscx", bufs=2, space="PSUM"))

    # constants: weights (fp32r, [128, KC*128] with k-chunk c at cols 128c),
    # biases [128, 1] (v/sv bias folded into obias), identity [128, 64]
    wt, bt = {}, {}

    def load_consts(names):
        # sync HWDGE queue for the simple 2D bias loads; weights keep the
        # known-good gpsimd SWDGE path (3D gather pattern).
        for p in names:
            wt[p] = const.tile([128, KC * 128], MM, tag=f"w_{p}", name=f"w_{p}")
            nc.gpsimd.dma_start(
                out=wt[p].rearrange("p (c n) -> p c n", n=128),
                in_=prm[f"W{p}"].rearrange("(c p) n -> p c n", p=128),
            )
            if p in ("v", "sv"):
                continue
            bt[p] = const.tile([128, 1], f32, tag=f"b_{p}", name=f"b_{p}")
            nc.sync.dma_start(out=bt[p][:], in_=prm[f"b{p}"][:, None])

    load_consts(["q", "k"])
    ident = const.tile([128, DH], f32, tag="ident", name="ident")
    nc.sync.dma_start(out=ident[:], in_=prm["ident"][:])
    identb = const.tile([128, DH], bf16, tag="identb", name="identb")
    nc.vector.tensor_copy(identb[:], ident[:])
    obias = const.tile([128, 8 * CW], f32, tag="obias", name="obias")
    nc.sync.dma_start(out=obias[:], in_=prm["obias"][:])

    # per-batch state handed from proj gen to attn gen
    projT = [None] * B   # dict p -> [128, S] tile (qT/kT/sqT/skT fp32r, vT/svT f32)
    vaug = [None] * B    # dict (br, h) -> [128, JC*65] fp32r tile

    def issue_x(b):
        # SWDGE (Pool) queue, parallel to the sync HWDGE queue; issued for
        # all batches upfront so the PE is never x-starved.
        xt = []
        for kc in range(KC):
            t = xpool.tile([128, S], MM, tag="xt", name="xt")
            nc.gpsimd.dma_start(
                out=t[:], in_=prm["xT"][128 * kc : 128 * (kc + 1), S * b : S * (b + 1)]
            )
            xt.append(t)
        return xt

    def gen_proj(b, xt):
        pj = {}
        for p in PROJ:
            pj[p] = popool.tile([128, S], MM, tag=f"pj_{p}", name=f"pj_{p}")
        projT[b] = pj
        # token-block-inner: consecutive matmuls share the stationary weight
        # chunk, so the PE pays one weight switch per NTB matmuls.
        for p in PROJ:
            ps = [ps_mm.tile([128, 512], f32, tag="pmm", name=f"ps_{p}")[:] for _ in range(NTB)]
            for kc in range(KC):
                for tb in range(NTB):
                    nc.tensor.matmul(
                        ps[tb],
                        wt[p][:, 128 * kc : 128 * (kc + 1)],
                        xt[kc][:, 512 * tb : 512 * (tb + 1)],
                        start=(kc == 0),
                        stop=(kc == KC - 1),
                    )
                yield
            for tb in range(NTB):
                if p in ("v", "sv"):
                    # bias folded into obias; cast f32 PSUM -> bf16 SBUF
                    nc.vector.tensor_copy(
                        pj[p][:, 512 * tb : 512 * (tb + 1)], ps[tb]
                    )
                else:
                    nc.vector.tensor_scalar_add(
                        pj[p][:, 512 * tb : 512 * (tb + 1)], ps[tb], bt[p][:]
                    )
        # v natural + ones column (denominator trick); PE transpose in bf16
        va = {}
        vaug[b] = va
        for br in range(2):
            vsrc = pj["v" if br == 0 else "sv"]
            for h in range(HPC):
                t = vapool.tile([128, JC * (DH + 1)], bf16, tag="vaug", name="vaug")
                va[br, h] = t
                tv = t.rearrange("p (c w) -> p c w", w=DH + 1)
                nc.vector.memset(tv[:, :, DH], 1.0)
                tp = ps_mm.tile([128, 512], bf16, tag="pmm", name="vtp")
                for jc in range(JC):
                    nc.tensor.transpose(
                        tp[:, DH * jc : DH * (jc + 1)],
                        vsrc[DH * h : DH * (h + 1), 128 * jc : 128 * (jc + 1)],
                        identb[DH * h : DH * (h + 1), :],
                    )
                yield
                nc.vector.tensor_copy(
                    tv[:, :, 0:DH], tp.rearrange("p (c w) -> p c w", w=DH)
                )
                yield

    def gen_attn(b):
        pj = projT[b]
        va = vaug[b]
        normed = {}
        for br in range(2):
            qT = pj["q" if br == 0 else "sq"]
            kT = pj["k" if br == 0 else "sk"]
            for ic in range(NIC):
                cx = {h: ps_cx.tile([128, 512], f32, tag="ctx", name=f"cx{h}") for h in range(HPC)}
                for jp in range(JC // 2):
                    pr = {}
                    for h in range(HPC):
                        sc = ps_sc.tile([128, 1024], f32, tag="sc", name="sc")
                        for half in range(2):
                            jc = 2 * jp + half
                            nc.tensor.matmul(
                                sc[:, 512 * half : 512 * (half + 1)],
                                kT[DH * h : DH * (h + 1), 128 * jc : 128 * (jc + 1)],
                                qT[DH * h : DH * (h + 1), 512 * ic : 512 * (ic + 1)],
                                start=True,
                                stop=True,
                            )
                        p = prpool.tile([128, 1024], bf16, tag="probs", name="probs")
                        nc.scalar.activation(p[:], sc[:], Act.Exp)
                        pr[h] = p
                    yield
                    for h in range(HPC):
                        for half in range(2):
                            jc = 2 * jp + half
                            nc.tensor.matmul(
                                cx[h][0 : DH + 1, :],
                                va[br, h][:, (DH + 1) * jc : (DH + 1) * (jc + 1)],
                                pr[h][:, 512 * half : 512 * (half + 1)],
                                start=(jc == 0),
                                stop=(jc == JC - 1),
                            )
                    yield
                for h in range(HPC):
                    rcp = rcpool.tile([1, 512], dt.float32, tag="rcp", name="rcp")
                    nc.vector.reciprocal(rcp[:], cx[h][DH : DH + 1, :])
                    rcpb = rcpool.tile([DH, 512], dt.float32, tag="rcpb", name="rcpb")
                    nc.gpsimd.partition_broadcast(rcpb[:], rcp[:])
                    nt = nmpool.tile([DH, 512], dt.float32, tag="normed", name="normed")
                    nc.vector.tensor_tensor(nt[:], cx[h][0:DH, :], rcpb[:], Alu.mult)
                    normed[br, h, ic] = nt
        outsb = oupool.tile([128, 8 * CW], dt.float32, tag="outsb", name="outsb")
        ov = outsb.rearrange("p (c w) -> p c w", w=CW)
        for h in range(HPC):
            tp = ps_mm.tile([128, 512], dt.float32, tag="pmm", name="otp")
            for ic in range(NIC):
                cb = nmpool.tile([DH, 512], dt.float32, tag="comb", name="comb", bufs=3)
                nc.vector.tensor_add(cb[:], normed[0, h, ic][:], normed[1, h, ic][:])
                for icc in range(4):
                    nc.tensor.transpose(
                        tp[:, DH * (4 * ic + icc) : DH * (4 * ic + icc + 1)],
                        cb[:, 128 * icc : 128 * (icc + 1)],
                        ident[0:DH, :],
                    )
            nc.vector.tensor_tensor(
                ov[:, :, DH * h : DH * (h + 1)],
                tp.rearrange("p (c w) -> p c w", w=DH),
                obias.rearrange("p (c w) -> p c w", w=CW)[:, :, DH * h : DH * (h + 1)],
                Alu.add,
            )
            yield
        nc.sync.dma_start(
            out=prm["out"][b].rearrange("(c p) d -> p c d", p=128),
            in_=outsb.rearrange("p (c d) -> p c d", d=CW)
        )

    # driver: software-pipeline proj(b+1) into attention(b)'s rounds so the
    # PE always has dense matmul work while ACT chews through the exps.
    # KREP repeats the whole pipeline in-NEFF (timing: slope vs rep count).
    first_rep = True
    for rep in range(int(os.environ.get("KREP", "1"))):
        # queue order on the Pool SWDGE ring is program order: batch-0 x
        # first, then the remaining weights, then the other batches' x.
        xts = [issue_x(0)]
        if first_rep:
            load_consts(["v", "sq", "sk", "sv"])
            first_rep = False
        xts += [issue_x(b) for b in range(1, B)]
        pgens = [gen_proj(b, xts[b]) for b in range(B)]
        for _ in pgens[0]:
            pass
        for b in range(B):
            pg = pgens[b + 1] if b + 1 < B else None
            for _ in gen_attn(b):
                if pg is not None:
                    for _ in range(int(os.environ.get("KPULL", "1"))):
                        if next(pg, "done") == "done":
                            pg = None
                            break
            while pg is not None and next(pg, "done") != "done":
                pass


def build_nc():
    nc = bacc.Bacc("TRN2", target_bir_lowering=False, debug=False)
    prm = {}
    prm["xT"] = nc.declare_dram_parameter("xT", [D, B * S], MM, isOutput=False)
    for p in PROJ:
        prm[f"W{p}"] = nc.declare_dram_parameter(f"W{p}", [D, CW], MM, isOutput=False)
        if p not in ("v", "sv"):
            prm[f"b{p}"] = nc.declare_dram_parameter(f"b{p}", [CW], dt.float32, isOutput=False)
    prm["ident"] = nc.declare_dram_parameter("ident", [128, DH], dt.float32, isOutput=False)
    prm["obias"] = nc.declare_dram_parameter("obias", [128, 8 * CW], dt.float32, isOutput=False)
    prm["out"] = nc.declare_dram_parameter("out", [B, S, CW], dt.float32, isOutput=True)

    from contextlib import ExitStack

    with tile.TileContext(nc) as tc:
        with ExitStack() as ctx:
            _emit(nc, tc, ctx, prm)
    nc.compile()
    return nc


def make_in_maps(hidden_states, Wq, bq, Wk, bk, Wv, bv, Wsq, bsq, Wsk, bsk, Wsv, bsv, attn_w):
    """Host-side sharding: slice per-head weight columns, fold scales."""
    f32 = np.float32
    mmnp = dt.np(MM)
    x = np.asarray(hidden_states, f32).reshape(B * S, D)
    xT = np.ascontiguousarray(x.T).astype(mmnp)
    a = np.asarray(attn_w, f32)
    e = np.exp(a - a.max())
    w = (e / e.sum()).astype(f32)
    sc = f32(1.0 / np.sqrt(DH))
    ident = np.tile(np.eye(DH, dtype=f32), (2, 1))

    full = {
        "q": (np.asarray(Wq, f32) * sc, np.asarray(bq, f32) * sc),
        "k": (np.asarray(Wk, f32), np.asarray(bk, f32)),
        "v": (np.asarray(Wv, f32) * w[0], None),
        "sq": (np.asarray(Wsq, f32) * sc, np.asarray(bsq, f32) * sc),
        "sk": (np.asarray(Wsk, f32), np.asarray(bsk, f32)),
        "sv": (np.asarray(Wsv, f32) * w[1], None),
    }
    # v/sv bias folds linearly through attention into a constant output bias
    comb_bias = (w[0] * np.asarray(bv, f32) + w[1] * np.asarray(bsv, f32)).astype(f32)
    in_maps = []
    for c in range(NCORES):
        cols = slice(CW * c, CW * (c + 1))
        m = {"xT": xT, "ident": ident}
        m["obias"] = np.ascontiguousarray(
            np.tile(comb_bias[cols], (128, 8)).astype(f32)
        )
        for p in PROJ:
            W, b = full[p]
            m[f"W{p}"] = np.ascontiguousarray(W[:, cols]).astype(mmnp)
            if b is not None:
                m[f"b{p}"] = np.ascontiguousarray(b[cols])
        in_maps.append(m)
    return in_maps


_NC_CACHE = {}


def get_nc():
    if "nc" not in _NC_CACHE:
        _NC_CACHE["nc"] = build_nc()
    return _NC_CACHE["nc"]


def kernel(**inputs):
    nc = get_nc()
    in_maps = make_in_maps(**inputs)
    res = run_bass_kernel_spmd(nc, in_maps, list(range(NCORES)))
    parts = [res.results[c]["out"] for c in range(NCORES)]
    return np.concatenate(parts, axis=2).astype(np.float32)

